# revision 1
# baseline (speedup 1.0000x reference)
"""MoE feed-forward (top-2 of 8 experts, SwiGLU) on 8 TRN2 NeuronCores.

Expert parallelism: core c owns expert c (its up/down projections). Every core
receives the full token set, computes the router for all tokens (router weights
replicated, with columns permuted so column 0 is the core's own expert), runs
its expert densely over all tokens, scales each token's output row by that
expert's renormalized top-2 gate (zero when the expert is not in the token's
top-2), and the host sums the 8 partial outputs.

Matmuls run in float32r (TF32-like PE mode, ~1e-4 rel err, 4x faster than
plain fp32). The gate is computed from raw logits: with m1 = max logit,
m2 = second max, gate_e = [l_e >= m2] * exp(l_e - m1) / (1 + exp(m2 - m1)),
which equals softmax-top2-renormalized exactly (the softmax normalizer
cancels).
"""

import numpy as np

E = 8
H = 1024
D = 1024
T = 2048
KT = H // 128          # 8 k-tiles over the contraction dim
CH = 512               # token chunk (matmul moving free dim)
NCH = T // CH          # 4 chunks
MT_PER_CH = CH // 128  # 4 token 128-tiles per chunk

_cache = {}


def _build():
    import concourse.bass as bass
    import concourse.mybir as mybir
    from concourse import bacc
    from concourse.tile import TileContext
    from concourse.masks import make_identity

    f32 = mybir.dt.float32
    f32r = mybir.dt.float32r
    AX = mybir.AxisListType
    ALU = mybir.AluOpType
    ACT = mybir.ActivationFunctionType

    nc = bacc.Bacc(None, target_bir_lowering=False)

    xt_d = nc.declare_dram_parameter("xt", [H, T], f32r, isOutput=False)
    rwt_d = nc.declare_dram_parameter("rwt", [H, E], f32r, isOutput=False)
    up_d = nc.declare_dram_parameter("up", [H, 2 * D], f32r, isOutput=False)
    dn_d = nc.declare_dram_parameter("dn", [D, H], f32r, isOutput=False)
    y_d = nc.declare_dram_parameter("y", [T, H], f32, isOutput=True)

    xt_v = xt_d.ap().rearrange("(k p) t -> p k t", p=128)    # [128, KT, T]
    rwt_v = rwt_d.ap().rearrange("(k p) e -> p k e", p=128)  # [128, KT, E]
    up_v = up_d.ap().rearrange("(k p) f -> p k f", p=128)    # [128, KT, 2D]
    dn_v = dn_d.ap().rearrange("(k p) h -> p k h", p=128)    # [128, KT, H]
    y_v = y_d.ap().rearrange("(m p) h -> m p h", p=128)      # [16, 128, H]

    with TileContext(nc) as tc:
        with tc.tile_pool(name="w", bufs=1) as wp, \
             tc.tile_pool(name="x", bufs=2) as xp, \
             tc.tile_pool(name="h", bufs=2) as hp, \
             tc.tile_pool(name="o", bufs=3) as op, \
             tc.tile_pool(name="g", bufs=4) as gp, \
             tc.tile_pool(name="ps", bufs=1, space="PSUM") as pp:

            ident = wp.tile([128, 128], f32)
            make_identity(nc, ident[:])

            rwt_sb = wp.tile([128, KT, E], f32r)
            nc.sync.dma_start(out=rwt_sb[:], in_=rwt_v[:])

            up_sb = []
            dn_sb = []
            for k in range(KT):
                u = wp.tile([128, 2 * D], f32r, tag=f"up{k}")
                nc.sync.dma_start(out=u[:], in_=up_v[:, k, :])
                up_sb.append(u)
                d = wp.tile([128, H], f32r, tag=f"dn{k}")
                nc.sync.dma_start(out=d[:], in_=dn_v[:, k, :])
                dn_sb.append(d)

            for ch in range(NCH):
                xt_c = xp.tile([128, KT, CH], f32r, tag="xt")
                for k in range(KT):
                    nc.sync.dma_start(out=xt_c[:, k, :],
                                      in_=xt_v[:, k, ch * CH:(ch + 1) * CH])

                # --- router: logitsT [E, CH] then transpose to [128, E] per m-tile
                plgT = pp.tile([E, CH], f32, tag="plgT")
                for k in range(KT):
                    nc.tensor.matmul(plgT[:], rwt_sb[:, k, :], xt_c[:, k, :],
                                     start=(k == 0), stop=(k == KT - 1))
                lgT = gp.tile([E, CH], f32, tag="lgT")
                nc.vector.tensor_copy(lgT[:], plgT[:])

                gates = []
                for mt in range(MT_PER_CH):
                    ptr = pp.tile([128, E], f32, tag="ptr")
                    nc.tensor.transpose(ptr[:], lgT[:, mt * 128:(mt + 1) * 128],
                                        ident[:E, :E])
                    lg = gp.tile([128, E], f32, tag="lg")
                    nc.vector.tensor_copy(lg[:], ptr[:])
                    m1n = gp.tile([128, 1], f32, tag="m1n")
                    nc.vector.tensor_reduce(m1n[:], lg[:], axis=AX.X, op=ALU.max,
                                            negate=True)        # -max
                    msk = gp.tile([128, E], f32, tag="msk")
                    # l + m1n == 0 -> top-1 mask
                    nc.vector.tensor_scalar(msk[:], lg[:], m1n[:], 0.0,
                                            op0=ALU.add, op1=ALU.is_equal)
                    t2 = gp.tile([128, E], f32, tag="t2")
                    nc.vector.scalar_tensor_tensor(t2[:], msk[:], -1e30, lg[:],
                                                   op0=ALU.mult, op1=ALU.add)
                    m2 = gp.tile([128, 1], f32, tag="m2")
                    nc.vector.tensor_reduce(m2[:], t2[:], axis=AX.X, op=ALU.max)
                    e1 = gp.tile([128, 1], f32, tag="e1")
                    nc.scalar.activation(e1[:], lg[:, 0:1], ACT.Exp, bias=m1n[:])
                    e2 = gp.tile([128, 1], f32, tag="e2")
                    nc.scalar.activation(e2[:], m2[:], ACT.Exp, bias=m1n[:])
                    den = gp.tile([128, 1], f32, tag="den")
                    nc.vector.tensor_scalar_add(den[:], e2[:], 1.0)
                    rec = gp.tile([128, 1], f32, tag="rec")
                    nc.vector.reciprocal(rec[:], den[:])
                    ind = gp.tile([128, 1], f32, tag="ind")
                    # l_0 >= m2  (my expert in top-2)
                    nc.vector.tensor_scalar(ind[:], lg[:, 0:1], m2[:], None,
                                            op0=ALU.is_ge)
                    g1 = gp.tile([128, 1], f32, tag="g1")
                    nc.vector.tensor_mul(g1[:], e1[:], rec[:])
                    gt = gp.tile([128, 1], f32, tag=f"gate{mt}")
                    nc.vector.tensor_mul(gt[:], g1[:], ind[:])
                    gates.append(gt)

                # --- up proj + SwiGLU -> hiddenT chunk [d-on-partition, tok]
                hi_c = hp.tile([128, KT, CH], f32r, tag="hi")
                for f in range(KT):
                    pa = pp.tile([128, CH], f32, tag="pup")
                    for k in range(KT):
                        nc.tensor.matmul(pa[:], up_sb[k][:, f * 128:(f + 1) * 128],
                                         xt_c[:, k, :],
                                         start=(k == 0), stop=(k == KT - 1))
                    pb = pp.tile([128, CH], f32, tag="pup")
                    for k in range(KT):
                        nc.tensor.matmul(pb[:],
                                         up_sb[k][:, D + f * 128:D + (f + 1) * 128],
                                         xt_c[:, k, :],
                                         start=(k == 0), stop=(k == KT - 1))
                    sl = gp.tile([128, CH], f32, tag="silu")
                    nc.scalar.activation(sl[:], pa[:], ACT.Silu)
                    nc.vector.tensor_mul(hi_c[:, f, :], sl[:], pb[:])

                # --- down proj + gate scale
                for mt in range(MT_PER_CH):
                    out_t = op.tile([128, H], f32, tag="out")
                    for n in range(2):
                        pd = pp.tile([128, 512], f32, tag="pdn")
                        for d in range(KT):
                            nc.tensor.matmul(pd[:],
                                             hi_c[:, d, mt * 128:(mt + 1) * 128],
                                             dn_sb[d][:, n * 512:(n + 1) * 512],
                                             start=(d == 0), stop=(d == KT - 1))
                        nc.vector.tensor_scalar_mul(out_t[:, n * 512:(n + 1) * 512],
                                                    pd[:], gates[mt][:])
                    nc.sync.dma_start(out=y_v[ch * MT_PER_CH + mt], in_=out_t[:])

    nc.finalize()
    return nc


def _get_nc():
    if "nc" not in _cache:
        _cache["nc"] = _build()
    return _cache["nc"]


def _in_maps(x, router_w, up_proj, down_proj):
    xt = np.ascontiguousarray(x.reshape(T, H).T).astype(np.float32)
    maps = []
    for c in range(E):
        perm = [c] + [e for e in range(E) if e != c]
        maps.append({
            "xt": xt,
            "rwt": np.ascontiguousarray(router_w[perm].T).astype(np.float32),
            "up": np.ascontiguousarray(up_proj[c]).astype(np.float32),
            "dn": np.ascontiguousarray(down_proj[c]).astype(np.float32),
        })
    return maps


def kernel(x, router_w, up_proj, down_proj):
    from concourse.bass_utils import run_bass_kernel_spmd

    nc = _get_nc()
    res = run_bass_kernel_spmd(nc, _in_maps(x, router_w, up_proj, down_proj),
                               list(range(E)))
    out = np.zeros((T, H), dtype=np.float64)
    for c in range(E):
        out += res.results[c]["y"]
    return out.astype(np.float32).reshape(1, T, H)
